# revision 44
# baseline (speedup 1.0000x reference)
"""BEiT attention block on 8 TRN2 NeuronCores, data-parallel over batch.

Full inputs -> kernel(**inputs) -> full output (16, 1025, 768) f32.

Per-core work: 2 batches of multi-head attention (N=1025 tokens, C=768,
H=12 heads, d=64) with a relative-position bias added to the logits.

Strategy (all matmul operands bf16, f32 PSUM accumulation):
  - host: transpose x -> xT chunks (kpb-major), pre-transpose weights,
    fold softmax into exp(s*scale) * exp(bias) with exp(bias^T)
    precomputed in bf16 (padded kpos rows are 0 so padded keys vanish).
  - device handles queries 0..1023 (two 512-wide chunks, full PSUM
    banks); the single leftover query (token 1024) is computed on host
    in f32 and patched into the output.
  - device per batch: qkvT = Wqk^T.T @ xT (q,k kept d-on-partitions),
    v in natural token-on-partitions layout with a ones column appended
    (PV then yields both attn@v and the softmax denominators).
  - scores computed transposed: sT[k, q] = k_h.T @ q_h, contraction d=64;
    the two heads of a pair sit at partitions 0-63 / 64-127 so their
    QK^T matmuls row-tile the PE array concurrently.
  - softmax reciprocal computed in-loop per chunk straight from the
    PSUM denominator row, broadcast back via DMA at the end, applied to
    unnormalized outputs, then proj per batch with bias-add on gpsimd.
"""

import numpy as np
import ml_dtypes

B = 16
N = 1025
C = 768
H = 12
D = 64
NCORES = 8
BPC = B // NCORES  # batches per core
NPAD = 1152        # padded key length: 9 * 128
KB = NPAD // 128   # key blocks
IB = C // 128      # input-channel blocks
NQ = 1024          # queries handled on device (token 1024 done on host)
QCS = [512, 512]          # query chunks (full PSUM banks)
QCO = [0, 512]
KCS = [512, 512]          # k-projection chunks; token 1024's k comes from host
KCO = [0, 512]
SCALE = D ** -0.5
# minimax cubic p(z) ~ exp(z) on [-0.55, 0.55], factored a3*(z-R)*(z^2+S*z+T);
# exp(y)*EB computed as (p(y/4) * a3*EB^(1/4))^4 in one custom DVE op.
EXP_R = -1.658048394110858
EXP_S = 1.462861309003841
EXP_T = 3.672443055287797
EXP_A3 = 0.1641584267735188
BF16 = ml_dtypes.bfloat16

_cache: dict = {}


def _register_exp4():
    """Register the fused quartic-exp custom DVE op: out = (p(z)*Src1)^4
    with p monic-factored; Src1 carries a3*EB^(1/4)."""
    if "exp4" in _cache:
        return _cache["exp4"]
    import numpy as np
    from concourse import dve_ops
    from concourse.dve_spec import Spec, Src0, Src1, C0, C1, C2
    from concourse.dve_table_gen import dve_ver_for
    from concourse.dve_uop import DveOpSpec

    name = "EXP4_EB_ANT"
    for op in dve_ops.OPS:
        if op.name == name:
            _cache["exp4"] = op
            return op

    m5 = (((Src0 + C1) * Src0) + C2) * (Src0 - C0) * Src1
    p2 = m5 * m5
    body = p2 * p2

    def ref(in0, in1, s0, s1, imm2):
        m5 = ((((in0 + s1) * in0) + imm2) * (in0 - s0) * in1).astype(np.float32)
        p2 = (m5 * m5).astype(np.float32)
        return (p2 * p2).astype(np.float32)

    op = dve_ops.DveOp(name, Spec(body=body, reference=ref), subdim=False,
                       uops_sha={})
    row = dve_ops._CUSTOM_DVE_ROW_BASE + len(dve_ops.OPS)
    assert row < 0x20
    dve_ops.OPS.append(op)
    dve_ops._SUB_OPCODE_FOR_NAME[name] = row
    dve_ops.CUSTOM_DVE_SPECS[name] = op.spec
    ver = dve_ver_for("TRN2")
    spec = DveOpSpec(name=name, opcode=row,
                     uops=dve_ops.lower(op.spec, ver=ver),
                     rd1_en=dve_ops.has_src1(op.spec))
    op.uops_sha[ver] = spec.sha(ver)
    _cache["exp4"] = op
    return op


def _build():
    import concourse.bass as bass
    import concourse.mybir as mybir
    import concourse.tile as tile
    from concourse import bacc

    dt = mybir.dt
    f32 = dt.float32
    bf = dt.bfloat16
    AFT = mybir.ActivationFunctionType
    ALU = mybir.AluOpType

    exp4_op = _register_exp4()
    nc = bacc.Bacc("TRN2", target_bir_lowering=False, debug=False)

    xT_d = nc.declare_dram_parameter("xT", [BPC, KB, C, 128], bf, isOutput=False)
    wqk_d = nc.declare_dram_parameter("wqk", [C, 2 * C], bf, isOutput=False)
    wv_d = nc.declare_dram_parameter("wv", [C, C], bf, isOutput=False)
    wp_d = nc.declare_dram_parameter("wp", [C, C], bf, isOutput=False)
    qkb_d = nc.declare_dram_parameter("qkb", [128, 2 * IB], f32, isOutput=False)
    vb_d = nc.declare_dram_parameter("vb", [128, C], bf, isOutput=False)
    pb_d = nc.declare_dram_parameter("pb", [128, C], bf, isOutput=False)
    eb_d = nc.declare_dram_parameter("eb", [H, 2, NPAD, 512], bf, isOutput=False)
    out_d = nc.declare_dram_parameter("out", [BPC, N, C], bf, isOutput=True)
    v1_d = nc.declare_dram_parameter("v1", [BPC, 1, H, D], bf, isOutput=False)
    k1_d = nc.declare_dram_parameter("k1", [BPC, C, 1], bf, isOutput=False)

    with tile.TileContext(nc) as tc:
        from contextlib import ExitStack

        ctx = ExitStack()
        with ctx:
            consts = ctx.enter_context(tc.tile_pool(name="consts", bufs=1))
            persist = ctx.enter_context(tc.tile_pool(name="persist", bufs=1))

            # attention pools created up front so early-closing scopes
            # (wv, xT) sit above them on the pool stack (LIFO release)
            qk_pool = ctx.enter_context(tc.tile_pool(name="qk", bufs=3))
            eb_pool = ctx.enter_context(tc.tile_pool(name="ebp", bufs=4))
            ex_pool = ctx.enter_context(tc.tile_pool(name="exp", bufs=3))
            stg_pool = ctx.enter_context(tc.tile_pool(name="stg", bufs=2))
            qk_ps = ctx.enter_context(tc.tile_pool(name="qk_ps", bufs=2, space="PSUM"))
            s_ps = ctx.enter_context(tc.tile_pool(name="s_ps", bufs=4, space="PSUM"))
            o_ps = ctx.enter_context(tc.tile_pool(name="o_ps", bufs=2, space="PSUM"))

            # xT pool created first so scopes close LIFO (wv closes before it)
            xT_ctx = ExitStack()
            xT_pool = xT_ctx.enter_context(tc.tile_pool(name="xTp", bufs=1))

            # ---- v-projection weights + first x chunk up front ----
            vproj_ctx = ExitStack()
            wvp = vproj_ctx.enter_context(tc.tile_pool(name="wv_pool", bufs=1))
            wv_sb = wvp.tile([128, IB, C], bf)
            xT_sb = [xT_pool.tile([128, IB, N], bf, name=f"xT{b}")
                     for b in range(BPC)]

            # xT loads get the gpsimd DMA queue to themselves (in consumption
            # order, rate-matched to vproj); weights ride the sync queue

            def xT_chunk(b, kpb, ih):
                nc.gpsimd.dma_start(
                    xT_sb[b][:, 3 * ih:3 * (ih + 1),
                             kpb * 128:(kpb + 1) * 128],
                    xT_d.ap()[b][kpb][384 * ih:384 * (ih + 1), :]
                    .rearrange("(i p) c -> p i c", p=128))

            for ih in range(2):
                xT_chunk(0, 0, ih)
            # ib0 split in two halves so the first vproj matmuls start
            # as early as possible; rest as full rows
            for vc in range(2):
                nc.sync.dma_start(
                    wv_sb[:, 0, vc * 384:(vc + 1) * 384],
                    wv_d.ap()[0:128, vc * 384:(vc + 1) * 384])
            for ib in range(1, IB):
                nc.sync.dma_start(
                    wv_sb[:, ib, :], wv_d.ap()[ib * 128:(ib + 1) * 128, :])
            vb_sb = wvp.tile([128, C], bf)
            nc.sync.dma_start(vb_sb[:], vb_d.ap())

            # ---- persistent per-batch tensors; xT loaded kpb-major ----
            v_sb = []
            a_sb = []
            for b in range(BPC):
                for kpb in range(KB - 1):
                    if b == 0 and kpb == 0:
                        continue
                    for ih in range(2):  # split each chunk across 2 queues
                        xT_chunk(b, kpb, ih)
                # cols 0..63 = v, col 64 = ones (softmax denominator lands
                # in po row 64; PSUM partition access must be 32-aligned).
                # kb8 holds only key 1024 in row 0 (v from host, ones=1);
                # its PV matmul is a K=1 outer product reading row 0 only.
                v_sb.append(persist.tile([128, KB, H, D + 1], bf, name=f"v{b}"))
                nc.gpsimd.memset(v_sb[b][:, :KB - 1, :, D:], 1.0)
                nc.gpsimd.memset(v_sb[b][0:1, KB - 1, :, D:], 1.0)
                nc.sync.dma_start(v_sb[b][0:1, KB - 1, :, :D], v1_d.ap()[b])
                a_sb.append(persist.tile([128, IB, NQ], bf, name=f"a{b}"))

            sums_sb = [persist.tile([H, NQ], f32, name=f"sums{b}")
                       for b in range(BPC)]

            # ---- v projection (natural layout, + ones column);
            # psum reused from the scores pool ----
            if True:
                vps = s_ps
                for b in range(BPC):
                    for kpb in range(KB - 1):  # kb8 (token 1024) from host
                        for vc in range(2):
                            ps = vps.tile([128, 384], f32, name="st")
                            for ib in range(IB):
                                nc.tensor.matmul(
                                    ps[:],
                                    lhsT=xT_sb[b][:, ib, kpb * 128:
                                                 (kpb + 1) * 128],
                                    rhs=wv_sb[:, ib, vc * 384:(vc + 1) * 384],
                                    start=(ib == 0),
                                    stop=(ib == IB - 1),
                                )
                            nc.vector.tensor_add(
                                out=v_sb[b][:, kpb,
                                            6 * vc:6 * (vc + 1), :D],
                                in0=ps[:].rearrange(
                                    "p (h d) -> p h d", d=D),
                                in1=vb_sb[:, vc * 384:(vc + 1) * 384]
                                .rearrange("p (h d) -> p h d", d=D),
                            )
            vproj_ctx.close()

            # ---- remaining constants (needed from the attention phase on) ----
            wqk_sb = consts.tile([128, IB, 2 * C], bf)
            for ib in range(IB):
                nc.sync.dma_start(
                    wqk_sb[:, ib, :], wqk_d.ap()[ib * 128:(ib + 1) * 128, :])
            qkb_sb = consts.tile([128, 2 * IB], f32)
            nc.sync.dma_start(qkb_sb[:], qkb_d.ap())
            ones1 = consts.tile([1, 128], bf)
            nc.gpsimd.memset(ones1[:], 1.0)
            pb_sb = consts.tile([128, C], bf)
            nc.sync.dma_start(pb_sb[:], pb_d.ap())
            wp_sb = consts.tile([128, IB, C], bf)
            for ib in range(IB):
                nc.sync.dma_start(
                    wp_sb[:, ib, :], wp_d.ap()[ib * 128:(ib + 1) * 128, :])

            # ---- attention over head pairs ----
            recip_dram = nc.dram_tensor("recip_dram", [BPC * H, NQ], bf)
            HPL = H // 2
            rb_pool = None
            yt_pool = None

            def load_ebt(hp, qc):
                ebt = []
                for par in range(2):
                    h = 2 * hp + par
                    t = eb_pool.tile([128, KB, 512], bf, name="ebt")
                    for k0, k1 in ((0, 1), (1, 2), (2, 4), (4, 6), (6, 8)):
                        nc.sync.dma_start(
                            t[:, k0:k1, :],
                            eb_d.ap()[h][qc][k0 * 128:k1 * 128, :]
                            .rearrange("(kb p) q -> p kb q", p=128),
                        )
                    # kb8: only key 1024 is real; its eb row holds exp(bias)
                    # untransformed (for both pars)
                    nc.sync.dma_start(
                        t[0:1, KB - 1, :],
                        eb_d.ap()[h][qc][(KB - 1) * 128:
                                         (KB - 1) * 128 + 1, :],
                    )
                    ebt.append(t)
                return ebt

            def attn_unit(hp, qc, b, qk2, ebt):
                """scores + exp + PV for one (qc, b): the vector engine
                paces this (DVE exp4 ops), so kb0 is shifted to the
                scalar+gpsimd path for BOTH pars and kb8 (1 real key) uses
                plain scalar exp. kb8 goes first so its row-0 eb-mul sits
                ahead of the exp4s in the vector queue."""
                qcs, qco = QCS[qc], QCO[qc]
                ex = [ex_pool.tile([128, KB, 512], bf, name="ex")
                      for _ in range(2)]
                for kb in [KB - 1] + list(range(KB - 1)):
                    for par in range(2):
                        p0 = par * 64
                        st = s_ps.tile([128, 512], f32, name="st")
                        if kb == KB - 1:
                            # kb8 = single real key (token 1024): M=1
                            # scores row, tiny exp, tiny eb-mul; PV reads
                            # only row 0 via a K=1 outer product
                            nc.tensor.matmul(
                                st[0:1, :qcs],
                                lhsT=qk2[b][p0:p0 + 64, 1, NQ:NQ + 1],
                                rhs=qk2[b][p0:p0 + 64, 0, qco:qco + qcs],
                            )
                            nc.scalar.activation(
                                out=ex[par][0:1, kb, :qcs],
                                in_=st[0:1, :qcs],
                                func=AFT.Exp, scale=4.0,
                            )
                            nc.vector.tensor_mul(
                                out=ex[par][0:1, kb, :qcs],
                                in0=ex[par][0:1, kb, :qcs],
                                in1=ebt[par][0:1, kb, :qcs],
                            )
                            continue
                        nc.tensor.matmul(
                            st[:, :qcs],
                            lhsT=qk2[b][p0:p0 + 64, 1, kb * 128:(kb + 1) * 128],
                            rhs=qk2[b][p0:p0 + 64, 0, qco:qco + qcs],
                        )
                        if par == 0 and kb > 0:
                            nc.vector._custom_dve(
                                exp4_op,
                                out=ex[par][:, kb, :qcs],
                                in0=st[:, :qcs],
                                in1=ebt[par][:, kb, :qcs],
                                s0=EXP_R, s1=EXP_S, imm2=EXP_T,
                            )
                        else:
                            nc.scalar.activation(
                                out=ex[par][:, kb, :qcs],
                                in_=st[:, :qcs],
                                func=AFT.Exp, scale=4.0,
                            )
                            if kb < 4:
                                nc.gpsimd.tensor_mul(
                                    out=ex[par][:, kb, :qcs],
                                    in0=ex[par][:, kb, :qcs],
                                    in1=ebt[par][:, kb, :qcs],
                                )
                            elif kb == KB - 2:
                                nc.vector.tensor_mul(
                                    out=ex[par][:, 4:KB - 1, :qcs],
                                    in0=ex[par][:, 4:KB - 1, :qcs],
                                    in1=ebt[par][:, 4:KB - 1, :qcs],
                                )
                for par in range(2):
                    h = 2 * hp + par
                    po = o_ps.tile([D + 1, 512], f32, name="po")
                    for kb in [KB - 1] + list(range(KB - 1)):
                        rows = 1 if kb == KB - 1 else 128
                        nc.tensor.matmul(
                            po[:, :qcs],
                            lhsT=v_sb[b][:rows, kb, h, :],
                            rhs=ex[par][:rows, kb, :qcs],
                            start=(kb == KB - 1),
                            stop=(kb == KB - 2),
                        )
                    # denominator sits in po row 64 (32-aligned PSUM
                    # read); DMA remaps the partition into sums
                    stg = stg_pool.tile([65, 512], f32, name="stg")
                    nc.vector.tensor_copy(
                        out=stg[64:65, :qcs], in_=po[D:D + 1, :qcs])
                    nc.sync.dma_start(
                        sums_sb[b][h:h + 1, qco:qco + qcs],
                        stg[64:65, :qcs],
                    )
                    nc.scalar.activation(
                        out=a_sb[b][par * 64:(par + 1) * 64, hp,
                                    qco:qco + qcs],
                        in_=po[:D, :qcs],
                        func=AFT.Copy,
                    )

            def recip_half(b, half, rb, final=False):
                """reciprocal of one qc-half of the denominators +
                partition-broadcast via a DRAM round-trip. The final call
                spreads its broadcast across 4 engine DMA queues (exposed
                at the kernel tail); earlier calls ride the idle sync
                queue under plenty of attention cover."""
                o = half * 512
                recip_b = stg_pool.tile([H, 512], bf, name="stg")
                nc.vector.reciprocal_approx_fast(
                    out=sums_sb[b][:, o:o + 512],
                    in_=sums_sb[b][:, o:o + 512])
                nc.vector.tensor_copy(
                    out=recip_b[:], in_=sums_sb[b][:, o:o + 512])
                nc.sync.dma_start(
                    recip_dram.ap()[b * H:(b + 1) * H, o:o + 512],
                    recip_b[:])
                engines = ((nc.sync, nc.scalar, nc.gpsimd, nc.sync)
                           if final else (nc.sync,) * 4)
                base = recip_dram.ap()
                k = 0
                for par in range(2):
                    for it in range(2):
                        bcast = bass.AP(
                            tensor=base.tensor,
                            offset=(b * H + par + 6 * it) * NQ + o,
                            ap=[[0, 64], [2 * NQ, 3], [1, 512]],
                        )
                        engines[k].dma_start(
                            rb[par * 64:(par + 1) * 64,
                               3 * it:3 * (it + 1), o:o + 512], bcast)
                        k += 1

            def norm_muls(b, rb, half):
                nc.vector.tensor_mul(
                    out=a_sb[b][:, :, half * 512:(half + 1) * 512],
                    in0=a_sb[b][:, :, half * 512:(half + 1) * 512],
                    in1=rb[:, :, half * 512:(half + 1) * 512],
                )

            def proj_batch(b, qbs):
                for qb in qbs:
                    yt = yt_pool.tile([128, C], bf, name="yt")
                    for oc2 in range(2):
                        ps = qk_ps.tile([128, 384], f32, name="qkps")
                        # proj bias added on the PE via a K=1 ones-row
                        # matmul, so the psum drain is a plain copy
                        nc.tensor.matmul(
                            ps[:],
                            lhsT=ones1[:],
                            rhs=pb_sb[0:1, oc2 * 384:(oc2 + 1) * 384],
                            start=True, stop=False,
                        )
                        for ib in range(IB):
                            nc.tensor.matmul(
                                ps[:],
                                lhsT=a_sb[b][:, ib, qb * 128:(qb + 1) * 128],
                                rhs=wp_sb[:, ib, oc2 * 384:(oc2 + 1) * 384],
                                start=False,
                                stop=(ib == IB - 1),
                            )
                        nc.scalar.activation(
                            out=yt[:, oc2 * 384:(oc2 + 1) * 384],
                            in_=ps[:],
                            func=AFT.Copy,
                        )
                    # stores ride the scalar queue (ordered right after the
                    # drain), keeping the sync queue free for the
                    # reciprocal round-trip
                    for ih in range(2):
                        nc.scalar.dma_start(
                            out_d.ap()[b][qb * 128 + 64 * ih:
                                          qb * 128 + 64 * (ih + 1), :],
                            yt[64 * ih:64 * (ih + 1), :],
                        )

            for hp in range(H // 2):
                # q/k projection for this head pair, both batches.
                # q rows cover tokens 0..1023 (two 512 chunks); k rows cover
                # tokens 0..1024 (chunks 384/384/257). Bias/scale applied on
                # gpsimd (q side) and scalar (k side) to balance engines.
                qk2 = []
                for b in range(BPC):
                    t = qk_pool.tile([128, 2, NPAD], bf, name="qk2")
                    nc.gpsimd.memset(t[:, 1, N:], 0.0)
                    # k for token 1024 is computed on host
                    nc.sync.dma_start(
                        t[:, 1, NQ:N],
                        k1_d.ap()[b][hp * 128:(hp + 1) * 128])
                    for sec in range(2):  # 0 = q rows, 1 = k rows
                        ocb = sec * IB + hp
                        chunks = zip(QCO, QCS) if sec == 0 else zip(KCO, KCS)
                        for co, cs in chunks:
                            ps = qk_ps.tile([128, 512], f32, name="qkps")
                            for ib in range(IB):
                                nc.tensor.matmul(
                                    ps[:, :cs],
                                    lhsT=wqk_sb[:, ib, sec * C + hp * 128:
                                                sec * C + (hp + 1) * 128],
                                    rhs=xT_sb[b][:, ib, co:co + cs],
                                    start=(ib == 0),
                                    stop=(ib == IB - 1),
                                )
                            if sec == 0:
                                nc.scalar.activation(
                                    out=t[:, 0, co:co + cs],
                                    in_=ps[:, :cs],
                                    func=AFT.Identity,
                                    bias=qkb_sb[:, ocb:ocb + 1],
                                    scale=float(SCALE / 4),
                                )
                            else:
                                # k bias is structurally zero in BEiT
                                nc.scalar.activation(
                                    out=t[:, 1, co:co + cs],
                                    in_=ps[:, :cs],
                                    func=AFT.Copy,
                                )
                    qk2.append(t)

                if hp < HPL - 1:
                    for qc in range(2):
                        ebt = load_ebt(hp, qc)
                        for b in range(BPC):
                            attn_unit(hp, qc, b, qk2, ebt)
                else:
                    # last head pair: b-outer order so b0 finishes early and
                    # its normalize + projection interleave with b1's
                    # remaining attention in the PE queue, hiding the
                    # reciprocal round-trip latency entirely.
                    # x no longer needed; reuse its SBUF for the reciprocal
                    # broadcast + proj staging tiles.
                    xT_ctx.close()
                    rb_pool = ctx.enter_context(
                        tc.tile_pool(name="rb", bufs=2))
                    yt_pool = ctx.enter_context(
                        tc.tile_pool(name="yt", bufs=2))
                    ebt0 = load_ebt(hp, 0)
                    ebt1 = load_ebt(hp, 1)
                    rb0 = rb_pool.tile([128, IB, NQ], bf, name="rb")
                    rb1 = rb_pool.tile([128, IB, NQ], bf, name="rb")
                    attn_unit(hp, 0, 0, qk2, ebt0)
                    attn_unit(hp, 1, 0, qk2, ebt1)
                    recip_half(0, 0, rb0)
                    recip_half(0, 1, rb0)
                    attn_unit(hp, 0, 1, qk2, ebt0)
                    norm_muls(0, rb0, 0)
                    norm_muls(0, rb0, 1)
                    recip_half(1, 0, rb1)
                    proj_batch(0, range(0, 4))
                    attn_unit(hp, 1, 1, qk2, ebt1)
                    norm_muls(1, rb1, 0)
                    proj_batch(0, range(4, 8))
                    recip_half(1, 1, rb1, final=True)
                    norm_muls(1, rb1, 1)
                    proj_batch(1, range(0, 8))

    nc.compile()
    return nc


def _prepare_inputs(x, qkv_weight, q_bias, v_bias, rel_pos_table, proj_weight,
                    proj_bias, rel_pos_index):
    x = np.asarray(x, np.float32)
    qkv_weight = np.asarray(qkv_weight, np.float32)
    q_bias = np.asarray(q_bias, np.float32)
    v_bias = np.asarray(v_bias, np.float32)
    rel_pos_table = np.asarray(rel_pos_table, np.float32)
    proj_weight = np.asarray(proj_weight, np.float32)
    proj_bias = np.asarray(proj_bias, np.float32)
    rel_pos_index = np.asarray(rel_pos_index)

    wqk = np.ascontiguousarray(qkv_weight[:2 * C].T).astype(BF16)
    wv = np.ascontiguousarray(qkv_weight[2 * C:].T).astype(BF16)
    wp = np.ascontiguousarray(proj_weight.T).astype(BF16)

    qkb = np.concatenate([q_bias * np.float32(SCALE / 4), np.zeros(C, np.float32)])
    qkb = np.ascontiguousarray(qkb.reshape(2 * IB, 128).T)  # [128, 12]
    vb = np.ascontiguousarray(np.broadcast_to(v_bias, (128, C))).astype(BF16)
    pb = np.ascontiguousarray(np.broadcast_to(proj_bias, (128, C))).astype(BF16)

    # exp of transposed rel-pos bias for device queries 0..1023;
    # padded key rows = 0 so padded keys vanish from the softmax sums
    bias_qkh = rel_pos_table[rel_pos_index.reshape(-1)].reshape(N, N, H)
    bT = np.zeros((H, N, NQ), np.float64)
    bT[:, :, :] = bias_qkh.transpose(2, 1, 0)[:, :, :NQ]
    ebt = np.zeros((H, NPAD, NQ), BF16)
    for h in range(H):
        if h % 2 == 0:
            ebt[h, :N, :] = (EXP_A3 * np.exp(bT[h] / 4)).astype(BF16)
        else:
            ebt[h, :N, :] = np.exp(bT[h]).astype(BF16)
    # key-1024 row (kb8 row 0) and key block 0 (rows 0-127) go through a
    # plain scalar-exp path on device for BOTH pars, so they hold exp(bias)
    # untransformed (kb0 is shifted off the DVE to rebalance engines)
    ebt[:, NQ, :] = np.exp(bT[:, NQ, :]).astype(BF16)
    ebt[:, 0:128, :] = np.exp(bT[:, 0:128, :]).astype(BF16)
    ebc = np.zeros((H, 2, NPAD, 512), BF16)
    for qc in range(2):
        ebc[:, qc, :, :] = ebt[:, :, QCO[qc]:QCO[qc] + 512]

    # v and k for token 1024, computed on host: (B, C)
    wv_f32 = qkv_weight[2 * C:]
    v1024 = x[:, NQ] @ wv_f32.T + v_bias  # (B, C)
    k1024 = x[:, NQ] @ qkv_weight[C:2 * C].T  # (B, C); k bias is zero

    in_maps = []
    for core in range(NCORES):
        xb = x[core * BPC:(core + 1) * BPC]
        xTf = np.zeros((BPC, C, NPAD), np.float32)
        xTf[:, :, :N] = xb.transpose(0, 2, 1)
        # kpb-major chunks: [BPC, KB, C, 128]
        xT = np.ascontiguousarray(
            xTf.reshape(BPC, C, KB, 128).transpose(0, 2, 1, 3)).astype(BF16)
        v1c = np.ascontiguousarray(
            v1024[core * BPC:(core + 1) * BPC].reshape(BPC, 1, H, D)
        ).astype(BF16)
        k1c = np.ascontiguousarray(
            k1024[core * BPC:(core + 1) * BPC].reshape(BPC, C, 1)
        ).astype(BF16)
        in_maps.append({
            "xT": xT, "wqk": wqk, "wv": wv, "wp": wp,
            "qkb": qkb, "vb": vb, "pb": pb, "eb": ebc, "v1": v1c,
            "k1": k1c,
        })
    return in_maps


def _host_last_query(x, qkv_weight, q_bias, v_bias, rel_pos_table,
                     proj_weight, proj_bias, rel_pos_index):
    """f32 attention output for query token 1024 (all batches): the device
    covers queries 0..1023; this row is patched in on the host."""
    x = np.asarray(x, np.float32)
    qkv_weight = np.asarray(qkv_weight, np.float32)
    q_bias = np.asarray(q_bias, np.float32)
    v_bias = np.asarray(v_bias, np.float32)
    rel_pos_table = np.asarray(rel_pos_table, np.float32)
    proj_weight = np.asarray(proj_weight, np.float32)
    proj_bias = np.asarray(proj_bias, np.float32)
    rel_pos_index = np.asarray(rel_pos_index)

    QI = NQ  # token index 1024
    hd = D
    wq = qkv_weight[:C]
    wk = qkv_weight[C:2 * C]
    wv = qkv_weight[2 * C:]
    q = x[:, QI] @ wq.T + q_bias              # (B, C)
    k = x @ wk.T                              # (B, N, C)
    v = x @ wv.T + v_bias                     # (B, N, C)
    qh = q.reshape(B, H, hd)
    kh = k.reshape(B, N, H, hd).transpose(0, 2, 1, 3)   # (B, H, N, d)
    vh = v.reshape(B, N, H, hd).transpose(0, 2, 1, 3)
    s = np.einsum('bhd,bhnd->bhn', qh * np.float32(SCALE), kh)
    bias = rel_pos_table[rel_pos_index[QI]].T            # (H, N)
    s = s + bias[None]
    s = s - s.max(axis=-1, keepdims=True)
    e = np.exp(s)
    attn = e / e.sum(axis=-1, keepdims=True)
    o = np.einsum('bhn,bhnd->bhd', attn, vh).reshape(B, C)
    return o @ proj_weight.T + proj_bias      # (B, C)


def kernel(**inputs) -> np.ndarray:
    from concourse.bass_utils import run_bass_kernel_spmd

    if "nc" not in _cache:
        _cache["nc"] = _build()
    nc = _cache["nc"]

    in_maps = _prepare_inputs(**inputs)
    trace = bool(_cache.get("trace", False))
    res = run_bass_kernel_spmd(nc, in_maps, core_ids=list(range(NCORES)),
                               trace=trace)
    _cache["last_results"] = res
    out = np.concatenate([r["out"] for r in res.results], axis=0)
    out = out.astype(np.float32)
    out[:, NQ, :] = _host_last_query(**inputs)
    return out

